# revision 25
# baseline (speedup 1.0000x reference)
"""Multi-head attention Trainium2 Bass kernel (bf16, DP4 x TP2, pipelined).

Problem: B=4, S=2048, H=16, DH=64, D=1024, fp32 inputs.
  q/k/v = hidden @ W{q,k,v}.T + b; scores = q k^T / 8; probs = softmax;
  ctx = probs v; out = ctx @ Wo.T + bo.

Sharding: batch data-parallel x head tensor-parallel. Core c owns batch
c//2 and heads 8*(c%2)..+8 (feature slice of 512). Host sums the 2
partial output projections per batch and adds bo.

v3 design (vs v2 baseline at ~400us):
  - ctx pair column-tiled: heads 2hp/2hp+1 run concurrently as PE
    col-tiles (0,0)/(0,64) with M=64 each -> 213ns/chunk instead of
    427, psum ctx accumulator is ONE bank [128, 512].
  - softmax denominator no longer rides a 65th Vaug column: per-chunk
    col-tiled ones-matmul pair accumulates pre-broadcast denominators
    in psum dn [128, 512] (den_h0 rows 0:64, den_h1 rows 64:128);
    normalize = copy + reciprocal_approx_fast + one tensor_tensor.
  - phase 0: single e-outer pass with 8 open psum groups (Q/K fc0 all
    four 512-token spans), paced by the hT DMA; biases DMA'd before wo.
  - fillers yield 1 matmul per step; hp3 gets 2 steps/chunk to drain
    the out-projection.
  - last unit (3,3) normalizes per q-half so outproj(3) overlaps.
"""
import numpy as np

import concourse.bass as bass
import concourse.tile as tile
from concourse import bacc, mybir
from concourse import bass_utils

F32 = mybir.dt.float32
F16 = mybir.dt.float16
BF16 = mybir.dt.bfloat16
EXP = mybir.ActivationFunctionType.Exp
ADD = mybir.AluOpType.add
MULT = mybir.AluOpType.mult

B = 4
S = 2048
D = 1024
NCORES = 8
P = 128          # partitions
EC = D // P      # 8 e-chunks
KC = S // P      # 16 kt chunks
F = 512          # per-core feature slice (8 heads x 64)
FC = F // P      # 4 f-chunks == head-pairs
NHP = 4          # head pairs per core
QS = 512         # q span per attention unit
NQS = S // QS    # 4 q spans


def build_nc(dbg=False):
    nc = bacc.Bacc("TRN2", target_bir_lowering=False, debug=False,
                   enable_asserts=True, num_devices=NCORES)

    hT = nc.dram_tensor("ht", [P, EC * S], BF16, kind="ExternalInput").ap()
    wq = nc.dram_tensor("wq", [P, EC * F], BF16, kind="ExternalInput").ap()
    wk = nc.dram_tensor("wk", [P, EC * F], BF16, kind="ExternalInput").ap()
    wv = nc.dram_tensor("wv", [P, EC * F], BF16, kind="ExternalInput").ap()
    wo = nc.dram_tensor("wo", [P, FC * D], BF16, kind="ExternalInput").ap()
    bq = nc.dram_tensor("bq", [P, FC], F32, kind="ExternalInput").ap()
    bk = nc.dram_tensor("bk", [P, FC], F32, kind="ExternalInput").ap()
    bv = nc.dram_tensor("bv", [1, F], F32, kind="ExternalInput").ap()
    out = nc.dram_tensor("out", [S, D], BF16, kind="ExternalOutput").ap()
    if dbg:
        d_acc = nc.dram_tensor("d_acc", [P, 2 * QS], F32,
                               kind="ExternalOutput").ap()
        d_rb = nc.dram_tensor("d_rb", [P, QS], F32,
                              kind="ExternalOutput").ap()
        d_psc = nc.dram_tensor("d_psc", [P, QS], F32,
                               kind="ExternalOutput").ap()
        d_ctxn = nc.dram_tensor("d_ctxn", [P, FC * S], BF16,
                                kind="ExternalOutput").ap()

    with tile.TileContext(nc) as tc:
        with (
            tc.tile_pool(name="const", bufs=1) as cpool,
            tc.tile_pool(name="wts", bufs=1) as wpool,
            tc.tile_pool(name="big", bufs=1) as bigp,
            tc.tile_pool(name="probs", bufs=4) as probsp,
            tc.tile_pool(name="rbp", bufs=2) as rbp,
            tc.tile_pool(name="ostage", bufs=2) as ostage,
            tc.tile_pool(name="pgrp", bufs=2, space="PSUM") as pgrp,
        ):
            # ---- input DMAs: weights q/k first, then hT e-chunks, wv
            # behind each ht chunk, biases right after, wo last.
            wqT = wpool.tile([P, EC, F], BF16, tag="wqT")
            wkT = wpool.tile([P, EC, F], BF16, tag="wkT")
            hts = []
            for e in range(EC):
                ht_e = bigp.tile([P, S], BF16, tag=f"ht{e}", name=f"ht{e}")
                hts.append(ht_e)
            wvT = wpool.tile([P, EC, F], BF16, tag="wvT")
            woT = wpool.tile([P, FC, D], BF16, tag="woT")
            bq_t = cpool.tile([P, FC], F32, tag="bq")
            bk_t = cpool.tile([P, FC], F32, tag="bk")
            bv_row = cpool.tile([1, F], F32, tag="bvr")
            nc.sync.dma_start(bq_t[:], bq)
            nc.sync.dma_start(bk_t[:], bk)
            nc.sync.dma_start(bv_row[:], bv)
            for e in range(EC):
                nc.sync.dma_start(wqT[:, e, :], wq[:, e * F:(e + 1) * F])
                nc.sync.dma_start(wkT[:, e, :], wk[:, e * F:(e + 1) * F])
                nc.sync.dma_start(hts[e][:], hT[:, e * S:(e + 1) * S])
                nc.sync.dma_start(wvT[:, e, :], wv[:, e * F:(e + 1) * F])
            nc.sync.dma_start(woT[:], wo.rearrange("p (c d) -> p c d", c=FC))

            ones16 = cpool.tile([P, 64], BF16)
            nc.gpsimd.memset(ones16[:], 1.0)
            bv_b = cpool.tile([P, F], F32, tag="bvb")
            nc.gpsimd.partition_broadcast(bv_b[:], bv_row[0:1, :])

            qT = bigp.tile([P, FC, S], BF16, tag="qT")
            kT = bigp.tile([P, FC, S], BF16, tag="kT")
            # v[tok, chunk, head, 0:64] (no denominator column in v3)
            vaug = bigp.tile([P, KC, 2 * NHP, 64], BF16, tag="vaug")
            ctxn = bigp.tile([P, FC, S], BF16, tag="ctxn")

            def qk_close(g, fc, tt, is_q):
                if is_q:
                    nc.vector.tensor_scalar(
                        qT[:, fc, bass.ts(tt, 512)], g[:],
                        bq_t[:, fc:fc + 1], 0.125, ADD, MULT)
                else:
                    nc.vector.tensor_scalar_add(
                        kT[:, fc, bass.ts(tt, 512)], g[:],
                        bk_t[:, fc:fc + 1])

            # ---- filler generators: yield once per emitted matmul ----
            def qk_proj_steps(fc, units=None):
                """Q/K proj for f-chunk fc, one 512-col group at a time.

                K first: it is consumed at the very start of head-pair
                fc's first span (scores stationary)."""
                if units is None:
                    units = [(wkT, tt) for tt in range(4)]
                    units += [(wqT, tt) for tt in range(4)]
                for wT, tt in units:
                    g = pgrp.tile([P, 512], F32, tag="fill")
                    for e in range(EC):
                        nc.tensor.matmul(
                            g[:], wT[:, e, bass.ts(fc, P)],
                            hts[e][:, bass.ts(tt, 512)],
                            start=(e == 0), stop=(e == EC - 1))
                        yield
                    qk_close(g, fc, tt, wT is wqT)
                    yield

            def v_unit(tc_i):
                """V for kt-chunk tc_i: [tok, f] via ht-stationary matmuls."""
                pv = pgrp.tile([P, F], F32, tag="fill")
                for e in range(EC):
                    nc.tensor.matmul(
                        pv[:], hts[e][:, bass.ts(tc_i, P)], wvT[:, e, :],
                        start=(e == 0), stop=(e == EC - 1))
                nc.vector.tensor_tensor(
                    vaug[:, tc_i, :, :],
                    pv[:].rearrange("p (h f) -> p h f", h=2 * NHP),
                    bv_b[:].rearrange("p (h f) -> p h f", h=2 * NHP),
                    ADD)

            def outproj_st(qs, st):
                """Output projection for one 128-token chunk of span qs."""
                t0 = qs * QS + st * P
                ot = ostage.tile([P, D], BF16)
                for half in range(2):
                    po = pgrp.tile([P, 512], F32, tag="fill")
                    for fc in range(FC):
                        nc.tensor.matmul(
                            po[:], ctxn[:, fc, t0:t0 + P],
                            woT[:, fc, bass.ts(half, 512)],
                            start=(fc == 0), stop=(fc == FC - 1))
                        yield
                    nc.vector.tensor_copy(ot[:, bass.ts(half, 512)], po[:])
                    yield
                nc.sync.dma_start(out[t0:t0 + P, :], ot[:])

            def outproj_steps(qs):
                for st in range(QS // P):
                    yield from outproj_st(qs, st)

            # ---- phase 0: K fc0 (all spans) + Q fc0 tt0, e-outer, 5
            # open psum groups (pgrp holds the other 2 banks; 5+2+1<=8).
            # Q fc0 tt1-3 follow as the first fillers inside (0,0).
            p0_units = [(wkT, tt) for tt in range(4)] + [(wqT, 0)]
            with tc.tile_pool(name="ps_qk0", bufs=5, space="PSUM") as projp:
                grps = [projp.tile([P, 512], F32, tag="p0", name=f"p0{i}")
                        for i in range(5)]
                for e in range(EC):
                    for gi, (wT, tt) in enumerate(p0_units):
                        nc.tensor.matmul(
                            grps[gi][:], wT[:, e, 0:P],
                            hts[e][:, bass.ts(tt, 512)],
                            start=(e == 0), stop=(e == EC - 1))
                for gi, (wT, tt) in enumerate(p0_units):
                    qk_close(grps[gi], 0, tt, wT is wqT)

            # ---- attention ----
            with (
                tc.tile_pool(name="ps_scores", bufs=2, space="PSUM") as ps_s,
                tc.tile_pool(name="ps_ctx", bufs=1, space="PSUM") as ps_c,
                tc.tile_pool(name="ps_dn", bufs=1, space="PSUM") as ps_dn,
            ):
                filler_q = []

                def filler_step():
                    while filler_q:
                        try:
                            next(filler_q[0])
                            return True
                        except StopIteration:
                            filler_q.pop(0)
                    return False

                additions = {
                    (0, 0): [qk_proj_steps(0, [(wqT, 1), (wqT, 2),
                                               (wqT, 3)])],
                    (0, 1): [qk_proj_steps(1)],
                    (1, 0): [qk_proj_steps(2)],
                    (2, 0): [qk_proj_steps(3)],
                    (3, 1): [outproj_steps(0)],
                    (3, 2): [outproj_steps(1)],
                    (3, 3): [outproj_steps(2)],
                }

                def normalize(hp, q0, qn, psc, dn):
                    """ctxn[:, hp, q0:q0+qn] = psc * (1/den) for both heads.

                    psc: [128, QS] psum (h-even rows 0:64, h-odd 64:128),
                    dn: [128, QS] psum chunk-accumulated denominators,
                    pre-broadcast (den_h0 in rows 0:64, den_h1 in
                    64:128 via the [128, 64] ones stationary)."""
                    qoff = q0 % QS
                    rb = rbp.tile([P, qn], F32, tag="rb")
                    nc.vector.tensor_copy(rb[:], dn[:, qoff:qoff + qn])
                    nc.vector.reciprocal_approx_fast(rb[:], rb[:])
                    if dbg and hp == 0 and q0 == 0:
                        ac32 = rbp.tile([P, QS], F32, tag="dacc")
                        nc.vector.tensor_copy(ac32[:], dn[:, 0:QS])
                        nc.sync.dma_start(d_acc[:, 0:QS], ac32[:])
                        nc.sync.dma_start(d_rb, rb[:])
                        ps32 = rbp.tile([P, QS], F32, tag="dpsc")
                        nc.vector.tensor_copy(ps32[:], psc[:])
                        nc.sync.dma_start(d_psc, ps32[:])
                    nc.vector.tensor_tensor(
                        ctxn[:, hp, q0:q0 + qn],
                        psc[:, qoff:qoff + qn], rb[:], MULT)

                for hp in range(NHP):
                    for qs in range(NQS):
                        if qs == 0 and hp >= 1:
                            # safety barrier: this head-pair's Q/K filler
                            # projections MUST be fully emitted before its
                            # scores (PE executes in program order; Tile
                            # does not catch the reversed-order hazard)
                            while filler_q:
                                try:
                                    next(filler_q[0])
                                except StopIteration:
                                    filler_q.pop(0)
                        filler_q.extend(additions.get((hp, qs), []))
                        q0 = qs * QS
                        psc = ps_c.tile([P, QS], F32, tag="ctx")
                        dn = ps_dn.tile([P, QS], F32, tag="dn")
                        prev = None
                        for c in range(KC):
                            if hp == 0 and qs == 0:
                                v_unit(c)
                            pss = ps_s.tile([P, 2, QS], F32, tag="scores")
                            # packed pair: heads 2hp (rows 0:64) and 2hp+1
                            # (rows 64:128) run concurrently as PE row-tiles
                            nc.tensor.matmul(
                                pss[:, 0, :],
                                kT[0:64, hp, bass.ts(c, P)],
                                qT[0:64, hp, q0:q0 + QS],
                                start=True, stop=True)
                            nc.tensor.matmul(
                                pss[:, 1, :],
                                kT[64:128, hp, bass.ts(c, P)],
                                qT[64:128, hp, q0:q0 + QS],
                                start=True, stop=True)
                            pr = probsp.tile([P, 2, QS], BF16)
                            nc.scalar.activation(pr[:], pss[:], EXP)
                            if prev is not None:
                                pv_, cc = prev
                                # col-tiled pairs: head 2hp in PE cols
                                # 0:64, head 2hp+1 in cols 64:128,
                                # concurrent. ctx then denominators (the
                                # ones stationary makes every output
                                # partition the kt partition-sum).
                                nc.tensor.matmul(
                                    psc[0:64, :],
                                    vaug[:, cc, 2 * hp, :],
                                    pv_[:, 0, :],
                                    start=(cc == 0), stop=False,
                                    tile_position=(0, 0))
                                nc.tensor.matmul(
                                    psc[64:128, :],
                                    vaug[:, cc, 2 * hp + 1, :],
                                    pv_[:, 1, :],
                                    start=(cc == 0), stop=False,
                                    tile_position=(0, 64))
                                nc.tensor.matmul(
                                    dn[0:64, :], ones16[:],
                                    pv_[:, 0, :],
                                    start=(cc == 0), stop=False,
                                    tile_position=(0, 0))
                                nc.tensor.matmul(
                                    dn[64:128, :], ones16[:],
                                    pv_[:, 1, :],
                                    start=(cc == 0), stop=False,
                                    tile_position=(0, 64))
                            prev = (pr, c)
                            for _ in range(3 if hp == 3 else 2):
                                filler_step()
                        pv_, cc = prev
                        nc.tensor.matmul(
                            psc[0:64, :], vaug[:, cc, 2 * hp, :],
                            pv_[:, 0, :], start=False, stop=True,
                            tile_position=(0, 0))
                        nc.tensor.matmul(
                            psc[64:128, :], vaug[:, cc, 2 * hp + 1, :],
                            pv_[:, 1, :], start=False, stop=True,
                            tile_position=(0, 64))
                        nc.tensor.matmul(
                            dn[0:64, :], ones16[:], pv_[:, 0, :],
                            start=False, stop=True, tile_position=(0, 0))
                        nc.tensor.matmul(
                            dn[64:128, :], ones16[:], pv_[:, 1, :],
                            start=False, stop=True, tile_position=(0, 64))

                        if hp == 3 and qs == 3:
                            # split normalize per q-half so outproj(3)
                            # token-chunks start while the 2nd half norms
                            for half in range(2):
                                normalize(hp, q0 + 256 * half, 256, psc, dn)
                                for st in (2 * half, 2 * half + 1):
                                    for _ in outproj_st(3, st):
                                        pass
                        else:
                            normalize(hp, q0, QS, psc, dn)

                # drain remaining fillers (tail of outproj qs2)
                while filler_q:
                    try:
                        next(filler_q[0])
                    except StopIteration:
                        filler_q.pop(0)
                if dbg:
                    nc.sync.dma_start(
                        d_ctxn, ctxn[:].rearrange("p a b -> p (a b)"))

    nc.compile()
    return nc


_NC_CACHE = None


def build_in_maps(hidden_states, Wq, bq, Wk, bk, Wv, bv, Wo):
    hid = np.asarray(hidden_states, np.float32)
    Wq = np.asarray(Wq, np.float32)
    Wk = np.asarray(Wk, np.float32)
    Wv = np.asarray(Wv, np.float32)
    Wo = np.asarray(Wo, np.float32)

    in_maps = []
    for c in range(NCORES):
        b = c // 2
        fs = (c % 2) * F
        sl = slice(fs, fs + F)
        hTb = hid[b].T  # [D, S]
        hTb = hTb.reshape(EC, P, S).transpose(1, 0, 2).reshape(P, EC * S)
        wqT = Wq[sl].T.reshape(EC, P, F).transpose(1, 0, 2).reshape(P, -1)
        wkT = Wk[sl].T.reshape(EC, P, F).transpose(1, 0, 2).reshape(P, -1)
        wvT = Wv[sl].T.reshape(EC, P, F).transpose(1, 0, 2).reshape(P, -1)
        # Wo[:, sl] is [D, F]; transpose -> [F, D] = [f, fo], f-chunked
        woT = Wo[:, sl].T.reshape(FC, P, D).transpose(1, 0, 2).reshape(P, -1)
        in_maps.append({
            "ht": to_bf16(hTb),
            "wq": to_bf16(wqT),
            "wk": to_bf16(wkT),
            "wv": to_bf16(wvT),
            "wo": to_bf16(woT),
            "bq": np.ascontiguousarray(
                np.asarray(bq, np.float32)[sl].reshape(FC, P).T),
            "bk": np.ascontiguousarray(
                np.asarray(bk, np.float32)[sl].reshape(FC, P).T),
            "bv": np.ascontiguousarray(
                np.asarray(bv, np.float32)[sl].reshape(1, F)),
        })
    return in_maps


def to_bf16(a):
    import ml_dtypes
    return np.ascontiguousarray(a.astype(ml_dtypes.bfloat16))


def kernel(hidden_states, Wq, bq, Wk, bk, Wv, bv, Wo, bo):
    global _NC_CACHE
    if _NC_CACHE is None:
        _NC_CACHE = build_nc()
    nc = _NC_CACHE

    in_maps = build_in_maps(hidden_states, Wq, bq, Wk, bk, Wv, bv, Wo)

    try:
        res = bass_utils.run_bass_kernel_spmd(nc, in_maps,
                                              core_ids=list(range(NCORES)))
    except Exception:
        # transient device flake: retry once
        res = bass_utils.run_bass_kernel_spmd(nc, in_maps,
                                              core_ids=list(range(NCORES)))
    bo = np.asarray(bo, dtype=np.float32)
    full = np.empty((B, S, D), dtype=np.float32)
    for b in range(B):
        full[b] = res.results[2 * b]["out"].astype(np.float32)
        full[b] += res.results[2 * b + 1]["out"]
        full[b] += bo
    return full


# revision 27
# speedup vs baseline: 1.3236x; 1.3236x over previous
"""Multi-head attention Trainium2 Bass kernel (bf16, DP4 x TP2, pipelined).

Problem: B=4, S=2048, H=16, DH=64, D=1024, fp32 inputs.
  q/k/v = hidden @ W{q,k,v}.T + b; scores = q k^T / 8; probs = softmax;
  ctx = probs v; out = ctx @ Wo.T + bo.

Sharding: batch data-parallel x head tensor-parallel. Core c owns batch
c//2 and heads 8*(c%2)..+8 (feature slice of 512). Host sums the 2
partial output projections per batch and adds bo.

v3 design (vs v2 baseline at ~400us):
  - ctx pair column-tiled: heads 2hp/2hp+1 run concurrently as PE
    col-tiles (0,0)/(0,64) with M=64 each -> 213ns/chunk instead of
    427, psum ctx accumulator is ONE bank [128, 512].
  - softmax denominator no longer rides a 65th Vaug column: per-chunk
    col-tiled ones-matmul pair accumulates pre-broadcast denominators
    in psum dn [128, 512] (den_h0 rows 0:64, den_h1 rows 64:128);
    normalize = copy + reciprocal_approx_fast + one tensor_tensor.
  - phase 0: single e-outer pass with 8 open psum groups (Q/K fc0 all
    four 512-token spans), paced by the hT DMA; biases DMA'd before wo.
  - fillers yield 1 matmul per step; hp3 gets 2 steps/chunk to drain
    the out-projection.
  - last unit (3,3) normalizes per q-half so outproj(3) overlaps.
"""
import numpy as np

import concourse.bass as bass
import concourse.tile as tile
from concourse import bacc, mybir
from concourse import bass_utils

F32 = mybir.dt.float32
F16 = mybir.dt.float16
BF16 = mybir.dt.bfloat16
EXP = mybir.ActivationFunctionType.Exp
ADD = mybir.AluOpType.add
MULT = mybir.AluOpType.mult

B = 4
S = 2048
D = 1024
NCORES = 8
P = 128          # partitions
EC = D // P      # 8 e-chunks
KC = S // P      # 16 kt chunks
F = 512          # per-core feature slice (8 heads x 64)
FC = F // P      # 4 f-chunks == head-pairs
NHP = 4          # head pairs per core
QS = 512         # q span per attention unit
NQS = S // QS    # 4 q spans


def build_nc(dbg=False):
    nc = bacc.Bacc("TRN2", target_bir_lowering=False, debug=False,
                   enable_asserts=True, num_devices=NCORES)

    hT = nc.dram_tensor("ht", [P, EC * S], BF16, kind="ExternalInput").ap()
    wq = nc.dram_tensor("wq", [P, EC * F], BF16, kind="ExternalInput").ap()
    wk = nc.dram_tensor("wk", [P, EC * F], BF16, kind="ExternalInput").ap()
    wv = nc.dram_tensor("wv", [P, EC * F], BF16, kind="ExternalInput").ap()
    wo = nc.dram_tensor("wo", [P, FC * D], BF16, kind="ExternalInput").ap()
    bq = nc.dram_tensor("bq", [P, FC], F32, kind="ExternalInput").ap()
    bk = nc.dram_tensor("bk", [P, FC], F32, kind="ExternalInput").ap()
    bv = nc.dram_tensor("bv", [1, F], F32, kind="ExternalInput").ap()
    out = nc.dram_tensor("out", [S, D], BF16, kind="ExternalOutput").ap()
    if dbg:
        d_acc = nc.dram_tensor("d_acc", [P, 2 * QS], F32,
                               kind="ExternalOutput").ap()
        d_rb = nc.dram_tensor("d_rb", [P, QS], F32,
                              kind="ExternalOutput").ap()
        d_psc = nc.dram_tensor("d_psc", [P, QS], F32,
                               kind="ExternalOutput").ap()
        d_ctxn = nc.dram_tensor("d_ctxn", [P, FC * S], BF16,
                                kind="ExternalOutput").ap()

    with tile.TileContext(nc) as tc:
        with (
            tc.tile_pool(name="const", bufs=1) as cpool,
            tc.tile_pool(name="wts", bufs=1) as wpool,
            tc.tile_pool(name="big", bufs=1) as bigp,
            tc.tile_pool(name="probs", bufs=6) as probsp,
            tc.tile_pool(name="accp", bufs=2) as accp,
            tc.tile_pool(name="rbp", bufs=2) as rbp,
            tc.tile_pool(name="ostage", bufs=2) as ostage,
            tc.tile_pool(name="pgrp", bufs=2, space="PSUM") as pgrp,
        ):
            # ---- input DMAs: weights q/k first, then hT e-chunks, wv
            # behind each ht chunk, biases right after, wo last.
            wqT = wpool.tile([P, EC, F], BF16, tag="wqT")
            wkT = wpool.tile([P, EC, F], BF16, tag="wkT")
            hts = []
            for e in range(EC):
                ht_e = bigp.tile([P, S], BF16, tag=f"ht{e}", name=f"ht{e}")
                hts.append(ht_e)
            wvT = wpool.tile([P, EC, F], BF16, tag="wvT")
            woT = wpool.tile([P, FC, D], BF16, tag="woT")
            bq_t = cpool.tile([P, FC], F32, tag="bq")
            bk_t = cpool.tile([P, FC], F32, tag="bk")
            bv_row = cpool.tile([1, F], F32, tag="bvr")
            nc.sync.dma_start(bq_t[:], bq)
            nc.sync.dma_start(bk_t[:], bk)
            nc.sync.dma_start(bv_row[:], bv)
            for e in range(EC):
                nc.sync.dma_start(wqT[:, e, :], wq[:, e * F:(e + 1) * F])
                nc.sync.dma_start(wkT[:, e, :], wk[:, e * F:(e + 1) * F])
                nc.sync.dma_start(hts[e][:], hT[:, e * S:(e + 1) * S])
                nc.sync.dma_start(wvT[:, e, :], wv[:, e * F:(e + 1) * F])
            nc.sync.dma_start(woT[:], wo.rearrange("p (c d) -> p c d", c=FC))

            ones16 = cpool.tile([P, 64], BF16)
            nc.gpsimd.memset(ones16[:], 1.0)
            bv_b = cpool.tile([P, F], F32, tag="bvb")
            nc.gpsimd.partition_broadcast(bv_b[:], bv_row[0:1, :])

            qT = bigp.tile([P, FC, S], BF16, tag="qT")
            kT = bigp.tile([P, FC, S], BF16, tag="kT")
            # v[tok, chunk, head, 0:64] (no denominator column in v3)
            vaug = bigp.tile([P, KC, 2 * NHP, 64], BF16, tag="vaug")
            ctxn = bigp.tile([P, FC, S], BF16, tag="ctxn")

            def qk_close(g, fc, tt, is_q):
                if is_q:
                    nc.vector.tensor_scalar(
                        qT[:, fc, bass.ts(tt, 512)], g[:],
                        bq_t[:, fc:fc + 1], 0.125, ADD, MULT)
                else:
                    nc.vector.tensor_scalar_add(
                        kT[:, fc, bass.ts(tt, 512)], g[:],
                        bk_t[:, fc:fc + 1])

            # ---- filler generators: yield once per emitted matmul ----
            def qk_proj_steps(fc, units=None):
                """Q/K proj for f-chunk fc, one 512-col group at a time.

                K first: it is consumed at the very start of head-pair
                fc's first span (scores stationary)."""
                if units is None:
                    units = [(wkT, tt) for tt in range(4)]
                    units += [(wqT, tt) for tt in range(4)]
                for wT, tt in units:
                    g = pgrp.tile([P, 512], F32, tag="fill")
                    for e in range(EC):
                        nc.tensor.matmul(
                            g[:], wT[:, e, bass.ts(fc, P)],
                            hts[e][:, bass.ts(tt, 512)],
                            start=(e == 0), stop=(e == EC - 1))
                        yield
                    qk_close(g, fc, tt, wT is wqT)
                    yield

            def v_unit(tc_i):
                """V for kt-chunk tc_i: [tok, f] via ht-stationary matmuls."""
                pv = pgrp.tile([P, F], F32, tag="fill")
                for e in range(EC):
                    nc.tensor.matmul(
                        pv[:], hts[e][:, bass.ts(tc_i, P)], wvT[:, e, :],
                        start=(e == 0), stop=(e == EC - 1))
                nc.vector.tensor_tensor(
                    vaug[:, tc_i, :, :],
                    pv[:].rearrange("p (h f) -> p h f", h=2 * NHP),
                    bv_b[:].rearrange("p (h f) -> p h f", h=2 * NHP),
                    ADD)

            def outproj_st(qs, st):
                """Output projection for one 128-token chunk of span qs."""
                t0 = qs * QS + st * P
                ot = ostage.tile([P, D], BF16)
                for half in range(2):
                    po = pgrp.tile([P, 512], F32, tag="fill")
                    for fc in range(FC):
                        nc.tensor.matmul(
                            po[:], ctxn[:, fc, t0:t0 + P],
                            woT[:, fc, bass.ts(half, 512)],
                            start=(fc == 0), stop=(fc == FC - 1))
                        yield
                    nc.vector.tensor_copy(ot[:, bass.ts(half, 512)], po[:])
                    yield
                nc.sync.dma_start(out[t0:t0 + P, :], ot[:])

            def outproj_steps(qs):
                for st in range(QS // P):
                    yield from outproj_st(qs, st)

            # ---- phase 0: K fc0 (all spans) + Q fc0 tt0, e-outer, 5
            # open psum groups (pgrp holds the other 2 banks; 5+2+1<=8).
            # Q fc0 tt1-3 follow as the first fillers inside (0,0).
            p0_units = [(wkT, tt) for tt in range(4)] + [(wqT, 0)]
            with tc.tile_pool(name="ps_qk0", bufs=5, space="PSUM") as projp:
                grps = [projp.tile([P, 512], F32, tag="p0", name=f"p0{i}")
                        for i in range(5)]
                for e in range(EC):
                    for gi, (wT, tt) in enumerate(p0_units):
                        nc.tensor.matmul(
                            grps[gi][:], wT[:, e, 0:P],
                            hts[e][:, bass.ts(tt, 512)],
                            start=(e == 0), stop=(e == EC - 1))
                for gi, (wT, tt) in enumerate(p0_units):
                    qk_close(grps[gi], 0, tt, wT is wqT)

            # ---- attention ----
            with (
                tc.tile_pool(name="ps_scores", bufs=2, space="PSUM") as ps_s,
                tc.tile_pool(name="ps_ctx", bufs=1, space="PSUM") as ps_c,
                tc.tile_pool(name="ps_den", bufs=1, space="PSUM") as ps_d,
            ):
                filler_q = []

                def filler_step():
                    while filler_q:
                        try:
                            next(filler_q[0])
                            return True
                        except StopIteration:
                            filler_q.pop(0)
                    return False

                additions = {
                    (0, 0): [qk_proj_steps(0, [(wqT, 1), (wqT, 2),
                                               (wqT, 3)])],
                    (0, 1): [qk_proj_steps(1)],
                    (1, 0): [qk_proj_steps(2)],
                    (2, 0): [qk_proj_steps(3)],
                    (3, 1): [outproj_steps(0)],
                    (3, 2): [outproj_steps(1)],
                    (3, 3): [outproj_steps(2)],
                }

                def normalize(hp, q0, qn, psc, accs):
                    """ctxn[:, hp, q0:q0+qn] = psc * (1/den) for both heads.

                    psc: [128, QS] psum (h-even rows 0:64, h-odd 64:128),
                    accs: two [128, 2, QS] fp16 half-chunk-summed probs.
                    The [128, 64] ones stationary makes every den output
                    partition the kt partition-sum (pre-broadcast),
                    col-tiled per head, psum-accumulated over both accs."""
                    qoff = q0 % QS
                    dn = ps_d.tile([P, qn], F32, tag="den")
                    for ai, acc in enumerate(accs):
                        nc.tensor.matmul(
                            dn[0:64, :], ones16[:],
                            acc[:, 0, qoff:qoff + qn],
                            start=(ai == 0), stop=(ai == 1),
                            tile_position=(0, 0))
                        nc.tensor.matmul(
                            dn[64:128, :], ones16[:],
                            acc[:, 1, qoff:qoff + qn],
                            start=(ai == 0), stop=(ai == 1),
                            tile_position=(0, 64))
                    rb = rbp.tile([P, qn], F32, tag="rb")
                    nc.vector.tensor_copy(rb[:], dn[:])
                    nc.vector.reciprocal_approx_fast(rb[:], rb[:])
                    if dbg and hp == 0 and q0 == 0:
                        nc.sync.dma_start(d_rb, rb[:])
                        ps32 = rbp.tile([P, QS], F32, tag="dpsc")
                        nc.vector.tensor_copy(ps32[:], psc[:])
                        nc.sync.dma_start(d_psc, ps32[:])
                    nc.vector.tensor_tensor(
                        ctxn[:, hp, q0:q0 + qn],
                        psc[:, qoff:qoff + qn], rb[:], MULT)

                for hp in range(NHP):
                    for qs in range(NQS):
                        if qs == 0 and hp >= 1:
                            # safety barrier: this head-pair's Q/K filler
                            # projections MUST be fully emitted before its
                            # scores (PE executes in program order; Tile
                            # does not catch the reversed-order hazard)
                            while filler_q:
                                try:
                                    next(filler_q[0])
                                except StopIteration:
                                    filler_q.pop(0)
                        filler_q.extend(additions.get((hp, qs), []))
                        q0 = qs * QS
                        psc = ps_c.tile([P, QS], F32, tag="ctx")
                        accs = [accp.tile([P, 2, QS], F16, tag=f"acc{i}",
                                          name=f"acc{i}")
                                for i in range(2)]
                        prev = None
                        for c in range(KC):
                            if hp == 0 and qs == 0:
                                v_unit(c)
                            pss = ps_s.tile([P, 2, QS], F32, tag="scores")
                            # packed pair: heads 2hp (rows 0:64) and 2hp+1
                            # (rows 64:128) run concurrently as PE row-tiles
                            nc.tensor.matmul(
                                pss[:, 0, :],
                                kT[0:64, hp, bass.ts(c, P)],
                                qT[0:64, hp, q0:q0 + QS],
                                start=True, stop=True)
                            nc.tensor.matmul(
                                pss[:, 1, :],
                                kT[64:128, hp, bass.ts(c, P)],
                                qT[64:128, hp, q0:q0 + QS],
                                start=True, stop=True)
                            pr = probsp.tile([P, 2, QS], BF16)
                            nc.scalar.activation(pr[:], pss[:], EXP)
                            acc = accs[c % 2]
                            if c < 2:
                                nc.vector.tensor_copy(acc[:], pr[:])
                            else:
                                nc.vector.tensor_tensor(
                                    acc[:], acc[:], pr[:], ADD)
                            if prev is not None:
                                pv_, cc = prev
                                # col-tiled pairs: head 2hp in PE cols
                                # 0:64, head 2hp+1 in cols 64:128,
                                # concurrent. ctx then denominators (the
                                # ones stationary makes every output
                                # partition the kt partition-sum).
                                nc.tensor.matmul(
                                    psc[0:64, :],
                                    vaug[:, cc, 2 * hp, :],
                                    pv_[:, 0, :],
                                    start=(cc == 0), stop=False,
                                    tile_position=(0, 0))
                                nc.tensor.matmul(
                                    psc[64:128, :],
                                    vaug[:, cc, 2 * hp + 1, :],
                                    pv_[:, 1, :],
                                    start=(cc == 0), stop=False,
                                    tile_position=(0, 64))
                            prev = (pr, c)
                            for _ in range(3 if hp == 3 else 2):
                                filler_step()
                        pv_, cc = prev
                        nc.tensor.matmul(
                            psc[0:64, :], vaug[:, cc, 2 * hp, :],
                            pv_[:, 0, :], start=False, stop=True,
                            tile_position=(0, 0))
                        nc.tensor.matmul(
                            psc[64:128, :], vaug[:, cc, 2 * hp + 1, :],
                            pv_[:, 1, :], start=False, stop=True,
                            tile_position=(0, 64))

                        if hp == 3 and qs == 3:
                            # split normalize per q-half so outproj(3)
                            # token-chunks start while the 2nd half norms
                            for half in range(2):
                                normalize(hp, q0 + 256 * half, 256, psc, accs)
                                for st in (2 * half, 2 * half + 1):
                                    for _ in outproj_st(3, st):
                                        pass
                        else:
                            normalize(hp, q0, QS, psc, accs)

                # drain remaining fillers (tail of outproj qs2)
                while filler_q:
                    try:
                        next(filler_q[0])
                    except StopIteration:
                        filler_q.pop(0)
                if dbg:
                    nc.sync.dma_start(
                        d_ctxn, ctxn[:].rearrange("p a b -> p (a b)"))

    nc.compile()
    return nc


_NC_CACHE = None


def build_in_maps(hidden_states, Wq, bq, Wk, bk, Wv, bv, Wo):
    hid = np.asarray(hidden_states, np.float32)
    Wq = np.asarray(Wq, np.float32)
    Wk = np.asarray(Wk, np.float32)
    Wv = np.asarray(Wv, np.float32)
    Wo = np.asarray(Wo, np.float32)

    in_maps = []
    for c in range(NCORES):
        b = c // 2
        fs = (c % 2) * F
        sl = slice(fs, fs + F)
        hTb = hid[b].T  # [D, S]
        hTb = hTb.reshape(EC, P, S).transpose(1, 0, 2).reshape(P, EC * S)
        wqT = Wq[sl].T.reshape(EC, P, F).transpose(1, 0, 2).reshape(P, -1)
        wkT = Wk[sl].T.reshape(EC, P, F).transpose(1, 0, 2).reshape(P, -1)
        wvT = Wv[sl].T.reshape(EC, P, F).transpose(1, 0, 2).reshape(P, -1)
        # Wo[:, sl] is [D, F]; transpose -> [F, D] = [f, fo], f-chunked
        woT = Wo[:, sl].T.reshape(FC, P, D).transpose(1, 0, 2).reshape(P, -1)
        in_maps.append({
            "ht": to_bf16(hTb),
            "wq": to_bf16(wqT),
            "wk": to_bf16(wkT),
            "wv": to_bf16(wvT),
            "wo": to_bf16(woT),
            "bq": np.ascontiguousarray(
                np.asarray(bq, np.float32)[sl].reshape(FC, P).T),
            "bk": np.ascontiguousarray(
                np.asarray(bk, np.float32)[sl].reshape(FC, P).T),
            "bv": np.ascontiguousarray(
                np.asarray(bv, np.float32)[sl].reshape(1, F)),
        })
    return in_maps


def to_bf16(a):
    import ml_dtypes
    return np.ascontiguousarray(a.astype(ml_dtypes.bfloat16))


def kernel(hidden_states, Wq, bq, Wk, bk, Wv, bv, Wo, bo):
    global _NC_CACHE
    if _NC_CACHE is None:
        _NC_CACHE = build_nc()
    nc = _NC_CACHE

    in_maps = build_in_maps(hidden_states, Wq, bq, Wk, bk, Wv, bv, Wo)

    try:
        res = bass_utils.run_bass_kernel_spmd(nc, in_maps,
                                              core_ids=list(range(NCORES)))
    except Exception:
        # transient device flake: retry once
        res = bass_utils.run_bass_kernel_spmd(nc, in_maps,
                                              core_ids=list(range(NCORES)))
    bo = np.asarray(bo, dtype=np.float32)
    full = np.empty((B, S, D), dtype=np.float32)
    for b in range(B):
        full[b] = res.results[2 * b]["out"].astype(np.float32)
        full[b] += res.results[2 * b + 1]["out"]
        full[b] += bo
    return full


# revision 28
# speedup vs baseline: 1.3268x; 1.0025x over previous
"""Multi-head attention Trainium2 Bass kernel (bf16, DP4 x TP2, pipelined).

Problem: B=4, S=2048, H=16, DH=64, D=1024, fp32 inputs.
  q/k/v = hidden @ W{q,k,v}.T + b; scores = q k^T / 8; probs = softmax;
  ctx = probs v; out = ctx @ Wo.T + bo.

Sharding: batch data-parallel x head tensor-parallel. Core c owns batch
c//2 and heads 8*(c%2)..+8 (feature slice of 512). Host sums the 2
partial output projections per batch and adds bo.

v3 design (vs v2 baseline at ~400us):
  - ctx pair column-tiled: heads 2hp/2hp+1 run concurrently as PE
    col-tiles (0,0)/(0,64) with M=64 each -> 213ns/chunk instead of
    427, psum ctx accumulator is ONE bank [128, 512].
  - softmax denominator no longer rides a 65th Vaug column: per-chunk
    col-tiled ones-matmul pair accumulates pre-broadcast denominators
    in psum dn [128, 512] (den_h0 rows 0:64, den_h1 rows 64:128);
    normalize = copy + reciprocal_approx_fast + one tensor_tensor.
  - phase 0: single e-outer pass with 8 open psum groups (Q/K fc0 all
    four 512-token spans), paced by the hT DMA; biases DMA'd before wo.
  - fillers yield 1 matmul per step; hp3 gets 2 steps/chunk to drain
    the out-projection.
  - last unit (3,3) normalizes per q-half so outproj(3) overlaps.
"""
import numpy as np

import concourse.bass as bass
import concourse.tile as tile
from concourse import bacc, mybir
from concourse import bass_utils

F32 = mybir.dt.float32
F16 = mybir.dt.float16
BF16 = mybir.dt.bfloat16
EXP = mybir.ActivationFunctionType.Exp
ADD = mybir.AluOpType.add
MULT = mybir.AluOpType.mult

B = 4
S = 2048
D = 1024
NCORES = 8
P = 128          # partitions
EC = D // P      # 8 e-chunks
KC = S // P      # 16 kt chunks
F = 512          # per-core feature slice (8 heads x 64)
FC = F // P      # 4 f-chunks == head-pairs
NHP = 4          # head pairs per core
QS = 512         # q span per attention unit
NQS = S // QS    # 4 q spans


def build_nc(dbg=False):
    nc = bacc.Bacc("TRN2", target_bir_lowering=False, debug=False,
                   enable_asserts=True, num_devices=NCORES)

    hT = nc.dram_tensor("ht", [P, EC * S], BF16, kind="ExternalInput").ap()
    wq = nc.dram_tensor("wq", [P, EC * F], BF16, kind="ExternalInput").ap()
    wk = nc.dram_tensor("wk", [P, EC * F], BF16, kind="ExternalInput").ap()
    wv = nc.dram_tensor("wv", [P, EC * F], BF16, kind="ExternalInput").ap()
    wo = nc.dram_tensor("wo", [P, FC * D], BF16, kind="ExternalInput").ap()
    bq = nc.dram_tensor("bq", [P, FC], F32, kind="ExternalInput").ap()
    bk = nc.dram_tensor("bk", [P, FC], F32, kind="ExternalInput").ap()
    bv = nc.dram_tensor("bv", [1, F], F32, kind="ExternalInput").ap()
    out = nc.dram_tensor("out", [S, D], BF16, kind="ExternalOutput").ap()
    if dbg:
        d_acc = nc.dram_tensor("d_acc", [P, 2 * QS], F32,
                               kind="ExternalOutput").ap()
        d_rb = nc.dram_tensor("d_rb", [P, QS], F32,
                              kind="ExternalOutput").ap()
        d_psc = nc.dram_tensor("d_psc", [P, QS], F32,
                               kind="ExternalOutput").ap()
        d_ctxn = nc.dram_tensor("d_ctxn", [P, FC * S], BF16,
                                kind="ExternalOutput").ap()

    with tile.TileContext(nc) as tc:
        with (
            tc.tile_pool(name="const", bufs=1) as cpool,
            tc.tile_pool(name="wts", bufs=1) as wpool,
            tc.tile_pool(name="big", bufs=1) as bigp,
            tc.tile_pool(name="probs", bufs=6) as probsp,
            tc.tile_pool(name="accp", bufs=2) as accp,
            tc.tile_pool(name="rbp", bufs=2) as rbp,
            tc.tile_pool(name="ostage", bufs=2) as ostage,
            tc.tile_pool(name="pgrp", bufs=2, space="PSUM") as pgrp,
        ):
            # ---- input DMAs: weights q/k first, then hT e-chunks, wv
            # behind each ht chunk, biases right after, wo last.
            wqT = wpool.tile([P, EC, F], BF16, tag="wqT")
            wkT = wpool.tile([P, EC, F], BF16, tag="wkT")
            htss = bigp.tile([P, EC, S], BF16, tag="hts")
            hts = [htss[:, e, :] for e in range(EC)]
            wvT = wpool.tile([P, EC, F], BF16, tag="wvT")
            woT = wpool.tile([P, FC, D], BF16, tag="woT")
            bq_t = cpool.tile([P, FC], F32, tag="bq")
            bk_t = cpool.tile([P, FC], F32, tag="bk")
            bv_row = cpool.tile([1, F], F32, tag="bvr")
            nc.sync.dma_start(bq_t[:], bq)
            nc.sync.dma_start(bk_t[:], bk)
            nc.sync.dma_start(bv_row[:], bv)
            nc.sync.dma_start(wqT[:], wq.rearrange("p (e f) -> p e f", e=EC))
            nc.sync.dma_start(wkT[:], wk.rearrange("p (e f) -> p e f", e=EC))
            half = EC // 2
            nc.sync.dma_start(htss[:, 0:half, :],
                              hT.rearrange("p (e s) -> p e s", e=EC)[:, 0:half, :])
            nc.sync.dma_start(htss[:, half:EC, :],
                              hT.rearrange("p (e s) -> p e s", e=EC)[:, half:EC, :])
            nc.sync.dma_start(wvT[:], wv.rearrange("p (e f) -> p e f", e=EC))
            nc.sync.dma_start(woT[:], wo.rearrange("p (c d) -> p c d", c=FC))

            ones16 = cpool.tile([P, 64], BF16)
            nc.gpsimd.memset(ones16[:], 1.0)
            bv_b = cpool.tile([P, F], F32, tag="bvb")
            nc.gpsimd.partition_broadcast(bv_b[:], bv_row[0:1, :])

            qT = bigp.tile([P, FC, S], BF16, tag="qT")
            kT = bigp.tile([P, FC, S], BF16, tag="kT")
            # v[tok, chunk, head, 0:64] (no denominator column in v3)
            vaug = bigp.tile([P, KC, 2 * NHP, 64], BF16, tag="vaug")
            ctxn = bigp.tile([P, FC, S], BF16, tag="ctxn")

            def qk_close(g, fc, tt, is_q):
                if is_q:
                    nc.vector.tensor_scalar(
                        qT[:, fc, bass.ts(tt, 512)], g[:],
                        bq_t[:, fc:fc + 1], 0.125, ADD, MULT)
                else:
                    nc.vector.tensor_scalar_add(
                        kT[:, fc, bass.ts(tt, 512)], g[:],
                        bk_t[:, fc:fc + 1])

            # ---- filler generators: yield once per emitted matmul ----
            def qk_proj_steps(fc, units=None):
                """Q/K proj for f-chunk fc, one 512-col group at a time.

                K first: it is consumed at the very start of head-pair
                fc's first span (scores stationary)."""
                if units is None:
                    units = [(wkT, tt) for tt in range(4)]
                    units += [(wqT, tt) for tt in range(4)]
                for wT, tt in units:
                    g = pgrp.tile([P, 512], F32, tag="fill")
                    for e in range(EC):
                        nc.tensor.matmul(
                            g[:], wT[:, e, bass.ts(fc, P)],
                            hts[e][:, bass.ts(tt, 512)],
                            start=(e == 0), stop=(e == EC - 1))
                        yield
                    qk_close(g, fc, tt, wT is wqT)
                    yield

            def v_unit(tc_i):
                """V for kt-chunk tc_i: [tok, f] via ht-stationary matmuls."""
                pv = pgrp.tile([P, F], F32, tag="fill")
                for e in range(EC):
                    nc.tensor.matmul(
                        pv[:], hts[e][:, bass.ts(tc_i, P)], wvT[:, e, :],
                        start=(e == 0), stop=(e == EC - 1))
                nc.vector.tensor_tensor(
                    vaug[:, tc_i, :, :],
                    pv[:].rearrange("p (h f) -> p h f", h=2 * NHP),
                    bv_b[:].rearrange("p (h f) -> p h f", h=2 * NHP),
                    ADD)

            def outproj_st(qs, st):
                """Output projection for one 128-token chunk of span qs."""
                t0 = qs * QS + st * P
                ot = ostage.tile([P, D], BF16)
                for half in range(2):
                    po = pgrp.tile([P, 512], F32, tag="fill")
                    for fc in range(FC):
                        nc.tensor.matmul(
                            po[:], ctxn[:, fc, t0:t0 + P],
                            woT[:, fc, bass.ts(half, 512)],
                            start=(fc == 0), stop=(fc == FC - 1))
                        yield
                    nc.vector.tensor_copy(ot[:, bass.ts(half, 512)], po[:])
                    yield
                nc.sync.dma_start(out[t0:t0 + P, :], ot[:])

            def outproj_steps(qs):
                for st in range(QS // P):
                    yield from outproj_st(qs, st)

            # ---- phase 0: K fc0 (all spans) + Q fc0 tt0, e-outer, 5
            # open psum groups (pgrp holds the other 2 banks; 5+2+1<=8).
            # Q fc0 tt1-3 follow as the first fillers inside (0,0).
            p0_units = [(wkT, 0), (wqT, 0)]
            with tc.tile_pool(name="ps_qk0", bufs=2, space="PSUM") as projp:
                grps = [projp.tile([P, 512], F32, tag="p0", name=f"p0{i}")
                        for i in range(len(p0_units))]
                for e in range(EC):
                    for gi, (wT, tt) in enumerate(p0_units):
                        nc.tensor.matmul(
                            grps[gi][:], wT[:, e, 0:P],
                            hts[e][:, bass.ts(tt, 512)],
                            start=(e == 0), stop=(e == EC - 1))
                for gi, (wT, tt) in enumerate(p0_units):
                    qk_close(grps[gi], 0, tt, wT is wqT)

            # ---- attention ----
            with (
                tc.tile_pool(name="ps_scores", bufs=2, space="PSUM") as ps_s,
                tc.tile_pool(name="ps_ctx", bufs=1, space="PSUM") as ps_c,
                tc.tile_pool(name="ps_den", bufs=1, space="PSUM") as ps_d,
            ):
                filler_q = []

                def filler_step():
                    while filler_q:
                        try:
                            next(filler_q[0])
                            return True
                        except StopIteration:
                            filler_q.pop(0)
                    return False

                additions = {
                    (0, 0): [qk_proj_steps(0, [(wkT, 1), (wkT, 2),
                                               (wkT, 3), (wqT, 1),
                                               (wqT, 2), (wqT, 3)])],
                    (0, 1): [qk_proj_steps(1)],
                    (1, 0): [qk_proj_steps(2)],
                    (2, 0): [qk_proj_steps(3)],
                    (3, 1): [outproj_steps(0)],
                    (3, 2): [outproj_steps(1)],
                    (3, 3): [outproj_steps(2)],
                }

                def normalize(hp, q0, qn, psc, accs):
                    """ctxn[:, hp, q0:q0+qn] = psc * (1/den) for both heads.

                    psc: [128, QS] psum (h-even rows 0:64, h-odd 64:128),
                    accs: two [128, 2, QS] fp16 half-chunk-summed probs.
                    The [128, 64] ones stationary makes every den output
                    partition the kt partition-sum (pre-broadcast),
                    col-tiled per head, psum-accumulated over both accs."""
                    qoff = q0 % QS
                    dn = ps_d.tile([P, qn], F32, tag="den")
                    for ai, acc in enumerate(accs):
                        nc.tensor.matmul(
                            dn[0:64, :], ones16[:],
                            acc[:, 0, qoff:qoff + qn],
                            start=(ai == 0), stop=(ai == 1),
                            tile_position=(0, 0))
                        nc.tensor.matmul(
                            dn[64:128, :], ones16[:],
                            acc[:, 1, qoff:qoff + qn],
                            start=(ai == 0), stop=(ai == 1),
                            tile_position=(0, 64))
                    rb = rbp.tile([P, qn], F32, tag="rb")
                    nc.vector.tensor_copy(rb[:], dn[:])
                    nc.vector.reciprocal_approx_fast(rb[:], rb[:])
                    if dbg and hp == 0 and q0 == 0:
                        nc.sync.dma_start(d_rb, rb[:])
                        ps32 = rbp.tile([P, QS], F32, tag="dpsc")
                        nc.vector.tensor_copy(ps32[:], psc[:])
                        nc.sync.dma_start(d_psc, ps32[:])
                    nc.vector.tensor_tensor(
                        ctxn[:, hp, q0:q0 + qn],
                        psc[:, qoff:qoff + qn], rb[:], MULT)

                for hp in range(NHP):
                    for qs in range(NQS):
                        if qs == 0 and hp >= 1:
                            # safety barrier: this head-pair's Q/K filler
                            # projections MUST be fully emitted before its
                            # scores (PE executes in program order; Tile
                            # does not catch the reversed-order hazard)
                            while filler_q:
                                try:
                                    next(filler_q[0])
                                except StopIteration:
                                    filler_q.pop(0)
                        filler_q.extend(additions.get((hp, qs), []))
                        q0 = qs * QS
                        psc = ps_c.tile([P, QS], F32, tag="ctx")
                        accs = [accp.tile([P, 2, QS], F16, tag=f"acc{i}",
                                          name=f"acc{i}")
                                for i in range(2)]
                        prev = None
                        for c in range(KC):
                            if hp == 0 and qs == 0:
                                v_unit(c)
                            pss = ps_s.tile([P, 2, QS], F32, tag="scores")
                            # packed pair: heads 2hp (rows 0:64) and 2hp+1
                            # (rows 64:128) run concurrently as PE row-tiles
                            nc.tensor.matmul(
                                pss[:, 0, :],
                                kT[0:64, hp, bass.ts(c, P)],
                                qT[0:64, hp, q0:q0 + QS],
                                start=True, stop=True)
                            nc.tensor.matmul(
                                pss[:, 1, :],
                                kT[64:128, hp, bass.ts(c, P)],
                                qT[64:128, hp, q0:q0 + QS],
                                start=True, stop=True)
                            pr = probsp.tile([P, 2, QS], BF16)
                            nc.scalar.activation(pr[:], pss[:], EXP)
                            acc = accs[c % 2]
                            if c < 2:
                                nc.vector.tensor_copy(acc[:], pr[:])
                            else:
                                nc.vector.tensor_tensor(
                                    acc[:], acc[:], pr[:], ADD)
                            if prev is not None:
                                pv_, cc = prev
                                # col-tiled pairs: head 2hp in PE cols
                                # 0:64, head 2hp+1 in cols 64:128,
                                # concurrent. ctx then denominators (the
                                # ones stationary makes every output
                                # partition the kt partition-sum).
                                nc.tensor.matmul(
                                    psc[0:64, :],
                                    vaug[:, cc, 2 * hp, :],
                                    pv_[:, 0, :],
                                    start=(cc == 0), stop=False,
                                    tile_position=(0, 0))
                                nc.tensor.matmul(
                                    psc[64:128, :],
                                    vaug[:, cc, 2 * hp + 1, :],
                                    pv_[:, 1, :],
                                    start=(cc == 0), stop=False,
                                    tile_position=(0, 64))
                            prev = (pr, c)
                            nfill = 3 if (hp == 0 and qs == 0) else 2
                            for _ in range(nfill):
                                filler_step()
                        pv_, cc = prev
                        nc.tensor.matmul(
                            psc[0:64, :], vaug[:, cc, 2 * hp, :],
                            pv_[:, 0, :], start=False, stop=True,
                            tile_position=(0, 0))
                        nc.tensor.matmul(
                            psc[64:128, :], vaug[:, cc, 2 * hp + 1, :],
                            pv_[:, 1, :], start=False, stop=True,
                            tile_position=(0, 64))

                        if hp == 3 and qs == 3:
                            # split normalize per q-half so outproj(3)
                            # token-chunks start while the 2nd half norms
                            for half in range(2):
                                normalize(hp, q0 + 256 * half, 256, psc, accs)
                                for st in (2 * half, 2 * half + 1):
                                    for _ in outproj_st(3, st):
                                        pass
                        else:
                            normalize(hp, q0, QS, psc, accs)

                # drain remaining fillers (tail of outproj qs2)
                while filler_q:
                    try:
                        next(filler_q[0])
                    except StopIteration:
                        filler_q.pop(0)
                if dbg:
                    nc.sync.dma_start(
                        d_ctxn, ctxn[:].rearrange("p a b -> p (a b)"))

    nc.compile()
    return nc


_NC_CACHE = None


def build_in_maps(hidden_states, Wq, bq, Wk, bk, Wv, bv, Wo):
    hid = np.asarray(hidden_states, np.float32)
    Wq = np.asarray(Wq, np.float32)
    Wk = np.asarray(Wk, np.float32)
    Wv = np.asarray(Wv, np.float32)
    Wo = np.asarray(Wo, np.float32)

    in_maps = []
    for c in range(NCORES):
        b = c // 2
        fs = (c % 2) * F
        sl = slice(fs, fs + F)
        hTb = hid[b].T  # [D, S]
        hTb = hTb.reshape(EC, P, S).transpose(1, 0, 2).reshape(P, EC * S)
        wqT = Wq[sl].T.reshape(EC, P, F).transpose(1, 0, 2).reshape(P, -1)
        wkT = Wk[sl].T.reshape(EC, P, F).transpose(1, 0, 2).reshape(P, -1)
        wvT = Wv[sl].T.reshape(EC, P, F).transpose(1, 0, 2).reshape(P, -1)
        # Wo[:, sl] is [D, F]; transpose -> [F, D] = [f, fo], f-chunked
        woT = Wo[:, sl].T.reshape(FC, P, D).transpose(1, 0, 2).reshape(P, -1)
        in_maps.append({
            "ht": to_bf16(hTb),
            "wq": to_bf16(wqT),
            "wk": to_bf16(wkT),
            "wv": to_bf16(wvT),
            "wo": to_bf16(woT),
            "bq": np.ascontiguousarray(
                np.asarray(bq, np.float32)[sl].reshape(FC, P).T),
            "bk": np.ascontiguousarray(
                np.asarray(bk, np.float32)[sl].reshape(FC, P).T),
            "bv": np.ascontiguousarray(
                np.asarray(bv, np.float32)[sl].reshape(1, F)),
        })
    return in_maps


def to_bf16(a):
    import ml_dtypes
    return np.ascontiguousarray(a.astype(ml_dtypes.bfloat16))


def kernel(hidden_states, Wq, bq, Wk, bk, Wv, bv, Wo, bo):
    global _NC_CACHE
    if _NC_CACHE is None:
        _NC_CACHE = build_nc()
    nc = _NC_CACHE

    in_maps = build_in_maps(hidden_states, Wq, bq, Wk, bk, Wv, bv, Wo)

    try:
        res = bass_utils.run_bass_kernel_spmd(nc, in_maps,
                                              core_ids=list(range(NCORES)))
    except Exception:
        # transient device flake: retry once
        res = bass_utils.run_bass_kernel_spmd(nc, in_maps,
                                              core_ids=list(range(NCORES)))
    bo = np.asarray(bo, dtype=np.float32)
    full = np.empty((B, S, D), dtype=np.float32)
    for b in range(B):
        full[b] = res.results[2 * b]["out"].astype(np.float32)
        full[b] += res.results[2 * b + 1]["out"]
        full[b] += bo
    return full
